# revision 52
# baseline (speedup 1.0000x reference)
"""3-layer GraphSAGE (mean aggr) on Trainium2, 8-core SPMD, fp16 compute.

Push-style message passing (v4). The baseline pulled source features after
AllGathering h1 [10240,256] and t3 [10240,128]; in the cost model those two
AllGathers cost ~227us (15us constant + out_bytes/40GB/s each). Instead
each core computes *partial* aggregates of its local sources for every
destination block with one-hot matmuls in PSUM (the same trick L1 uses,
1/deg folded into the one-hot), writes them to DRAM, and the partials are
summed with ReduceScatter, whose output (and hence modeled cost) is 8x
smaller: ~31us and ~19us.

  - Nodes padded 10000 -> 10240, 80 blocks of 128, degree-balanced by a
    host-side permutation. Core r owns blocks [10r,10r+10).
  - L1 is pull-based from the replicated x input.
  - L2 push: per-out-edge rows of local h1 [1280,256] are dma_gathered
    (edges sorted by destination block, ~3 chunks of 128 per block), and
    each destination block's partial mean^T [2,128,128] accumulates in
    PSUM, staged to SBUF bf16, and written to p2 [80*256,128] in 8-block
    slabs. ReduceScatter(add) leaves each core its own blocks' mean2^T —
    already transposed and degree-normalized, feeding the layer matmul
    rhs directly. (A dma_scatter_add design was tried first: real HW
    races on duplicate rows within a call, and serializing wave-split
    calls plus zero-initializing the targets cost more than the PE work.)
  - The w2r.h1T half of h2 only needs local data: precomputed into SBUF
    while RS1 blocks the Pool queue, then added in PSUM via an identity
    matmul. h2 lives only in SBUF; t3 = h2@[w3l|w3r] as in the baseline.
  - L3 push: same per-block one-hot partials over gathered t3 rows
    (node-major [10240,64]), RS2 -> own t3 mean sums.
  - log_softmax batches the 10 Exp calls then a single Ln.

Collectives must live on the Pool queue (the only engine the neuron
backend accepts for CollectiveCompute), so they serialize with the
gathers; the RS1 window is filled with the h2root PE precompute.

Cost-model time 195936 ns vs 334559 ns for the AllGather baseline
(collectives 50.5us vs 226.6us). End-to-end vs the f32 reference:
rel err ~3.1e-3 (dominated by the fp8 h1 rows the L2 push gathers;
h1's root-term path stays bf16).
"""

import numpy as np
BF = np.float16

N_NODES = 10000
NPAD = 10240
NCORES = 8
P = 128
NB = 10                      # dst blocks per core
NBLK = NPAD // P             # 80
PER_CORE = NB * P            # 1280
D_IN, D_H1, D_H2, D_OUT = 128, 256, 1024, 64

GCAP = 32                    # max chunks per push gather call
SW = 8                       # dst blocks per staged partial write

_CACHE = {}
LAST_RESULTS = None          # test harness reads exec_time_ns from here


def _build(MC, CB):
    import os
    import concourse.bacc as bacc
    import concourse.mybir as mybir
    import concourse.tile as tile

    abl = set(os.environ.get("KABL", "").split(","))

    f32 = mybir.dt.float32
    bf16 = mybir.dt.float16
    f8 = mybir.dt.float8e4
    i16 = mybir.dt.int16
    nc = bacc.Bacc("TRN2", target_bir_lowering=False, debug=False,
                   num_devices=NCORES)

    NCHP = sum(CB)
    off = [0]
    for cb in CB:
        off.append(off[-1] + cb)
    # push gather calls: whole blocks, at most GCAP chunks per call
    calls = []
    g0 = 0
    for g in range(1, NBLK + 1):
        if g == NBLK or off[g + 1] - off[g0] > GCAP:
            calls.append((g0, g, off[g0], off[g]))
            g0 = g
    blk_call = {}
    for ci, (a, b, c0, c1) in enumerate(calls):
        for g in range(a, b):
            blk_call[g] = (ci, c0)
    if SW >= 8:
        slab_bounds = (list(range(0, NBLK - 8, SW))
                       + [NBLK - 8, NBLK - 4, NBLK])
    else:
        slab_bounds = list(range(0, NBLK, SW)) + [NBLK]
    slab_start = {}
    slab_end = {}
    for i in range(len(slab_bounds) - 1):
        for g in range(slab_bounds[i], slab_bounds[i + 1]):
            slab_start[g] = slab_bounds[i]
            slab_end[g] = slab_bounds[i + 1]

    xbf = nc.dram_tensor("xbf", [NPAD, D_IN], bf16, kind="ExternalInput")
    xownT = nc.dram_tensor("xownT", [P, PER_CORE], bf16, kind="ExternalInput")
    w1l = nc.dram_tensor("w1l", [D_IN, D_H1], bf16, kind="ExternalInput")
    w1r = nc.dram_tensor("w1r", [D_IN, D_H1], bf16, kind="ExternalInput")
    b1 = nc.dram_tensor("b1", [1, D_H1], bf16, kind="ExternalInput")
    b1t = nc.dram_tensor("b1t", [P, 2], f32, kind="ExternalInput")
    w2l = nc.dram_tensor("w2l", [D_H1, D_H2], bf16, kind="ExternalInput")
    w2r = nc.dram_tensor("w2r", [D_H1, D_H2], bf16, kind="ExternalInput")
    b2t = nc.dram_tensor("b2t", [P, 8], f32, kind="ExternalInput")
    w3lr = nc.dram_tensor("w3lr", [D_H2, P], bf16, kind="ExternalInput")
    b3pad = nc.dram_tensor("b3pad", [1, P], bf16, kind="ExternalInput")
    ident_in = nc.dram_tensor("ident_in", [P, P], bf16, kind="ExternalInput")
    iota_in = nc.dram_tensor("iota_in", [P, P], bf16, kind="ExternalInput")
    # L1 pull tables (per own dst block, MC chunks of global in-edges)
    gidx = nc.dram_tensor("gidx", [P, NB * MC * 8], i16, kind="ExternalInput")
    dstloc = nc.dram_tensor("dstloc", [P, NB * MC], f32, kind="ExternalInput")
    invdeg = nc.dram_tensor("invdeg", [P, NB * MC], f32, kind="ExternalInput")
    # push tables (local out-edges chunked by destination block)
    g2t = nc.dram_tensor("g2t", [P, NCHP * 8], i16, kind="ExternalInput")
    dstp = nc.dram_tensor("dstp", [P, NCHP], f32, kind="ExternalInput")
    invp = nc.dram_tensor("invp", [P, NCHP], f32, kind="ExternalInput")
    outp = nc.dram_tensor("out", [P, NB * D_OUT], f32,
                          kind="ExternalOutput")

    EXP = mybir.ActivationFunctionType.Exp
    CPY = mybir.ActivationFunctionType.Copy
    LN = mybir.ActivationFunctionType.Ln
    RELU = mybir.ActivationFunctionType.Relu
    EQ = mybir.AluOpType.is_equal
    MUL = mybir.AluOpType.mult
    SUB = mybir.AluOpType.subtract
    ADD = mybir.AluOpType.add
    MAX = mybir.AluOpType.max
    AXX = mybir.AxisListType.X

    with tile.TileContext(nc) as tc:
        with (
            tc.tile_pool(name="const", bufs=1) as cp,
            tc.tile_pool(name="gath1", bufs=4) as gp1,
            tc.tile_pool(name="gath2", bufs=3) as gp2,
            tc.tile_pool(name="gath3", bufs=3) as gp3,
            tc.tile_pool(name="oh", bufs=48) as ohp,
            tc.tile_pool(name="meant", bufs=12) as mtp,
            tc.tile_pool(name="ht", bufs=2) as htp,
            tc.tile_pool(name="stage", bufs=4) as stp,
            tc.tile_pool(name="small", bufs=6) as smp,
            tc.tile_pool(name="psA", bufs=2, space="PSUM") as psA,
            tc.tile_pool(name="psO", bufs=1, space="PSUM") as psO,
            tc.tile_pool(name="psT", bufs=5, space="PSUM") as psT,
            tc.tile_pool(name="dram", bufs=1, space="DRAM") as dram,
        ):
            # ---- constants (SP queue input DMAs; gidx/x first: L1 needs them)
            gidx_sb = cp.tile([P, NB * MC * 8], i16, tag="gidx")
            nc.sync.dma_start(gidx_sb[:], gidx[:])
            dstloc_sb = cp.tile([P, NB * MC], f32, tag="dstloc")
            nc.sync.dma_start(dstloc_sb[:], dstloc[:])
            invdeg_sb = cp.tile([P, NB * MC], f32, tag="invdeg")
            nc.sync.dma_start(invdeg_sb[:], invdeg[:])
            iota_t = cp.tile([P, P], bf16, tag="iota")
            nc.sync.dma_start(iota_t[:], iota_in[:])
            w1l_sb = cp.tile([P, D_H1], bf16, tag="w1l")
            nc.sync.dma_start(w1l_sb[:], w1l[:])
            w1r_sb = cp.tile([P, D_H1], bf16, tag="w1r")
            nc.sync.dma_start(w1r_sb[:], w1r[:])
            b1_sb = cp.tile([1, D_H1], bf16, tag="b1")
            nc.sync.dma_start(b1_sb[:], b1[:])
            b1t_sb = cp.tile([P, 2], f32, tag="b1t")
            nc.sync.dma_start(b1t_sb[:], b1t[:])
            xT_res = cp.tile([P, PER_CORE], bf16, tag="xT")
            nc.sync.dma_start(xT_res[:], xownT[:])
            ones_t = cp.tile([1, P], bf16, tag="ones")
            nc.vector.memset(ones_t[:], 1.0)

            g2_sb = cp.tile([P, NCHP * 8], i16, tag="g2")
            nc.sync.dma_start(g2_sb[:], g2t[:])
            dstp_sb = cp.tile([P, NCHP], f32, tag="dstp")
            nc.sync.dma_start(dstp_sb[:], dstp[:])
            invp_sb = cp.tile([P, NCHP], f32, tag="invp")
            nc.sync.dma_start(invp_sb[:], invp[:])
            ident_t = cp.tile([P, P], bf16, tag="ident")
            nc.sync.dma_start(ident_t[:], ident_in[:])
            w2l_sb = cp.tile([P, 2, D_H2], bf16, tag="w2l")
            nc.sync.dma_start(w2l_sb[:], w2l.rearrange("(s p) n -> p s n", p=P))
            w2r_sb = cp.tile([P, 2, D_H2], bf16, tag="w2r")
            nc.sync.dma_start(w2r_sb[:], w2r.rearrange("(s p) n -> p s n", p=P))
            b2t_sb = cp.tile([P, 8], f32, tag="b2t")
            nc.sync.dma_start(b2t_sb[:], b2t[:])
            w3lr_sb = cp.tile([P, 8, P], bf16, tag="w3lr")
            nc.sync.dma_start(w3lr_sb[:], w3lr.rearrange("(s p) n -> p s n", p=P))
            b3_sb = cp.tile([1, P], bf16, tag="b3")
            nc.sync.dma_start(b3_sb[:], b3pad[:])

            # resident cross-phase SBUF state
            h1T_res = cp.tile([P, 2, PER_CORE], bf16, tag="h1T")
            h2root = cp.tile([P, NB, D_H2], bf16, tag="h2root")
            r3_res = cp.tile([P, NB, D_OUT], f32, tag="r3")
            ystore = cp.tile([P, NB, D_OUT], f32, tag="ystore")
            negm_res = cp.tile([P, NB], f32, tag="negm")
            sstore = cp.tile([P, NB], f32, tag="sstore")
            obstore = cp.tile([P, NB, D_OUT], f32, tag="obstore")

            # ---- DRAM intermediates ----
            h1f8 = dram.tile([PER_CORE, D_H1], f8, tag="h1f8")
            t3_own = dram.tile([PER_CORE, P], bf16, tag="t3o")
            p2 = dram.tile([NBLK * D_H1, P], bf16, tag="p2")    # [80,2,128,128]
            p3 = dram.tile([NPAD, D_OUT], bf16, tag="p3")       # [80,128,64]
            msum2 = dram.tile([NB * D_H1, P], bf16, tag="msum2")
            t3sum = dram.tile([PER_CORE, D_OUT], bf16, tag="t3sum")

            # ================= Layer 1 (pull, as baseline) =================
            meanTs = []
            for b in range(NB if "l1" not in abl else 0):
                gath = gp1.tile([P, MC, D_IN], bf16, tag="gath")
                c0 = b * MC * 8
                nc.gpsimd.dma_gather(
                    gath[:], xbf[:], gidx_sb[:, c0:c0 + MC * 8],
                    MC * P, MC * P, D_IN, single_packet=False)
                agg = psA.tile([P, 2 * P], f32, tag="agg")
                for c in range(MC):
                    col = b * MC + c
                    oh = ohp.tile([P, P], bf16, tag="oh")
                    nc.vector.tensor_scalar(
                        oh[:], iota_t[:],
                        dstloc_sb[:, col:col + 1], invdeg_sb[:, col:col + 1],
                        EQ, MUL)
                    nc.tensor.matmul(agg[:, 0:P], gath[:, c, :], oh[:],
                                     start=(c == 0), stop=(c == MC - 1))
                meanT = mtp.tile([P, P], bf16, tag="meanT1")
                nc.vector.tensor_copy(meanT[:], agg[:, 0:P])

                xT = xT_res[:, b * P:(b + 1) * P]
                # node-major h1 -> DRAM (gather source for L2 push)
                op = psO.tile([P, D_H1], f32, tag="outp")
                nc.tensor.matmul(op[:], meanT[:], w1l_sb[:],
                                 start=True, stop=False)
                nc.tensor.matmul(op[:], xT, w1r_sb[:],
                                 start=False, stop=False)
                nc.tensor.matmul(op[:], ones_t[:], b1_sb[:],
                                 start=False, stop=True)
                h1blk = smp.tile([P, D_H1], f8, tag="hout")
                nc.scalar.activation(h1blk[:], op[:], RELU)
                nc.sync.dma_start(h1f8[b * P:(b + 1) * P, :], h1blk[:])
                meanTs.append(meanT)

            # transposed h1 (L2 root term): deferred out of the L1 loop so
            # the gather-bound L1 cadence isn't PE-limited; runs under the
            # L2-push gather window.
            for b in range(len(meanTs)):
                xT = xT_res[:, b * P:(b + 1) * P]
                for s in range(2):
                    tph = psT.tile([P, P], f32, tag="tp")
                    nc.tensor.matmul(tph[:], w1l_sb[:, s * P:(s + 1) * P],
                                     meanTs[b][:], start=True, stop=False)
                    nc.tensor.matmul(tph[:], w1r_sb[:, s * P:(s + 1) * P],
                                     xT, start=False, stop=True)
                    nc.scalar.activation(h1T_res[:, s, b * P:(b + 1) * P],
                                         tph[:], RELU,
                                         bias=b1t_sb[:, s:s + 1])

            # ====== L2 push: one-hot matmul partial aggregation ======
            def push_partials(src_ap, elem, dt, emit_block, pool):
                tiles = {}
                for ci, (a, b_, c0, c1) in enumerate(calls):
                    g = pool.tile([P, GCAP, elem], dt, tag=f"pg{elem}")
                    nc.gpsimd.dma_gather(
                        g[:, 0:c1 - c0, :], src_ap, g2_sb[:, c0 * 8:c1 * 8],
                        (c1 - c0) * P, (c1 - c0) * P, elem,
                        single_packet=False)
                    tiles[ci] = g
                for g in range(NBLK):
                    ci, cbase = blk_call[g]
                    emit_block(g, tiles[ci], cbase)

            def mk_oh(c):
                oh = ohp.tile([P, P], bf16, tag="oh")
                nc.vector.tensor_scalar(
                    oh[:], iota_t[:], dstp_sb[:, c:c + 1],
                    invp_sb[:, c:c + 1], EQ, MUL)
                return oh

            if "l2p" not in abl:
                stg2 = [None]

                def l2_block(g, gtile, cbase):
                    agg = psA.tile([P, 2 * P], f32, tag="agg")
                    nchunk = off[g + 1] - off[g]
                    ohs = [mk_oh(off[g] + i) for i in range(nchunk)]
                    for s in range(2):
                        for i in range(nchunk):
                            lc = off[g] + i - cbase
                            nc.tensor.matmul(
                                agg[:, s * P:(s + 1) * P],
                                gtile[:, lc, s * P:(s + 1) * P], ohs[i][:],
                                start=(i == 0), stop=(i == nchunk - 1))
                    j = g - slab_start[g]
                    if j == 0:
                        stg2_t = stp.tile([P, SW, 2, P], bf16, tag="stg2")
                        stg2[0] = stg2_t
                    nc.scalar.activation(stg2[0][:, j, 0, :],
                                         agg[:, 0:P], CPY)
                    nc.vector.tensor_copy(stg2[0][:, j, 1, :], agg[:, P:2 * P])
                    if g == slab_end[g] - 1:
                        gA = slab_start[g]
                        nc.sync.dma_start(
                            p2[gA * D_H1:(g + 1) * D_H1, :].rearrange(
                                "(b s p) f -> p b s f", s=2, p=P),
                            stg2[0][:, 0:g + 1 - gA, :, :])

                push_partials(h1f8[:], D_H1, f8, l2_block, gp2)

            if "noag" not in abl:
                nc.gpsimd.collective_compute(
                    "ReduceScatter", mybir.AluOpType.add,
                    replica_groups=[list(range(NCORES))],
                    ins=[p2.opt()], outs=[msum2.opt()])

            # w2r.h1T precompute fills the RS1 window (PE+DVE are idle there)
            for b in range(NB if "l2c" not in abl else 0):
                for s in range(8):
                    hp = psT.tile([P, P], f32, tag="tp")
                    for k in range(2):
                        nc.tensor.matmul(
                            hp[:], w2r_sb[:, k, s * P:(s + 1) * P],
                            h1T_res[:, k, b * P:(b + 1) * P],
                            start=(k == 0), stop=(k == 1))
                    nc.scalar.activation(
                        h2root[:, b, s * P:(s + 1) * P], hp[:], CPY)

            # ====== L2 compute + L3 transform (h2 lives only in SBUF) ======
            # RS1 output is mean2^T per block, degree-normalized: the layer
            # matmul rhs directly.
            meanT_all = cp.tile([P, NB, 2, P], bf16, tag="meanT2")
            HB = NB // 2
            nc.scalar.dma_start(
                meanT_all[:, 0:HB, :, :],
                msum2[0:HB * D_H1, :].rearrange("(b s p) f -> p b s f",
                                                s=2, p=P))
            nc.scalar.dma_start(
                meanT_all[:, HB:NB, :, :],
                msum2[HB * D_H1:NB * D_H1, :].rearrange(
                    "(b s p) f -> p b s f", s=2, p=P))

            for b in range(NB if "l2c" not in abl else 0):
                # h2^T slices: w2l half on PE + staged root half (added in
                # PSUM via identity matmul), relu split across Act/DVE
                hT = htp.tile([P, 8, P], bf16, tag="hT")
                for s in range(8):
                    hp = psT.tile([P, P], f32, tag="tp")
                    for k in range(2):
                        nc.tensor.matmul(
                            hp[:], w2l_sb[:, k, s * P:(s + 1) * P],
                            meanT_all[:, b, k, :], start=(k == 0), stop=False)
                    nc.tensor.matmul(hp[:], ident_t[:],
                                     h2root[:, b, s * P:(s + 1) * P],
                                     start=False, stop=True)
                    if s % 2 == 0:
                        nc.scalar.activation(hT[:, s, :], hp[:], RELU,
                                             bias=b2t_sb[:, s:s + 1])
                    else:
                        nc.vector.tensor_scalar(hT[:, s, :], hp[:],
                                                b2t_sb[:, s:s + 1], 0.0,
                                                ADD, MAX)
                # [t3 | r3] = h2 @ [w3l | w3r] + [0 | b3]
                trt = psO.tile([P, D_H1], f32, tag="outp")
                tr = trt[:, 0:P]
                for s in range(8):
                    nc.tensor.matmul(tr, hT[:, s, :], w3lr_sb[:, s, :],
                                     start=(s == 0), stop=False)
                nc.tensor.matmul(tr, ones_t[:], b3_sb[:],
                                 start=False, stop=True)
                t3blk = smp.tile([P, P], bf16, tag="t3blk")
                nc.vector.tensor_copy(t3blk[:, 0:D_OUT], trt[:, 0:D_OUT])
                nc.vector.memset(t3blk[:, D_OUT:P], 0.0)
                nc.vector.tensor_copy(r3_res[:, b, :], trt[:, D_OUT:P])
                nc.sync.dma_start(t3_own[b * P:(b + 1) * P, :], t3blk[:])

            # ====== L3 push: same one-hot partials over t3 rows ======
            if "l3p" not in abl:
                stg3 = [None]

                def l3_block(g, gtile, cbase):
                    agg = psA.tile([P, 2 * P], f32, tag="agg")
                    nchunk = off[g + 1] - off[g]
                    for i in range(nchunk):
                        c = off[g] + i
                        lc = c - cbase
                        oh = mk_oh(c)
                        nc.tensor.matmul(agg[:, 0:D_OUT], oh[:],
                                         gtile[:, lc, 0:D_OUT],
                                         start=(i == 0),
                                         stop=(i == nchunk - 1))
                    j = g - slab_start[g]
                    if j == 0:
                        stg3_t = stp.tile([P, SW, D_OUT], bf16, tag="stg3")
                        stg3[0] = stg3_t
                    if g % 2 == 0:
                        nc.scalar.activation(stg3[0][:, j, :],
                                             agg[:, 0:D_OUT], CPY)
                    else:
                        nc.vector.tensor_copy(stg3[0][:, j, :],
                                              agg[:, 0:D_OUT])
                    if g == slab_end[g] - 1:
                        gA = slab_start[g]
                        nc.sync.dma_start(
                            p3[gA * P:(g + 1) * P, :].rearrange(
                                "(p j) f -> p j f", p=P),
                            stg3[0][:, 0:g + 1 - gA, :])

                push_partials(t3_own[:], P, bf16, l3_block, gp3)

            if "noag" not in abl:
                nc.gpsimd.collective_compute(
                    "ReduceScatter", mybir.AluOpType.add,
                    replica_groups=[list(range(NCORES))],
                    ins=[p3.opt()], outs=[t3sum.opt()])

            # ================= L3 final: mean + root + log_softmax ==========
            t3s_sb = cp.tile([P, NB // 2, 2, D_OUT], bf16, tag="t3s")
            HS = NB // 4
            nc.scalar.dma_start(
                t3s_sb[:, 0:HS, :, :],
                t3sum[0:HS * 2 * P, :].rearrange("(s p j) f -> p s j f",
                                                 p=P, j=2))
            nc.scalar.dma_start(
                t3s_sb[:, HS:NB // 2, :, :],
                t3sum[HS * 2 * P:NB * P, :].rearrange(
                    "(s p j) f -> p s j f", p=P, j=2))
            for b in range(NB if "l3f" not in abl else 0):
                nc.vector.tensor_tensor(ystore[:, b, :],
                                        t3s_sb[:, b // 2, b % 2, :],
                                        r3_res[:, b, :], ADD)
                nc.vector.tensor_reduce(negm_res[:, b:b + 1],
                                        ystore[:, b, :], AXX, MAX, negate=True)
            for b in range(NB if "l3f" not in abl else 0):
                e = smp.tile([P, D_OUT], f32, tag="e")
                nc.scalar.activation(e[:], ystore[:, b, :], EXP,
                                     bias=negm_res[:, b:b + 1], scale=1.0)
                nc.vector.tensor_reduce(sstore[:, b:b + 1], e[:], AXX, ADD)
            ls = cp.tile([P, NB], f32, tag="ls")
            if "l3f" not in abl:
                nc.scalar.activation(ls[:], sstore[:], LN)
            for b in range(NB if "l3f" not in abl else 0):
                nc.vector.tensor_scalar(obstore[:, b, :], ystore[:, b, :],
                                        negm_res[:, b:b + 1], ls[:, b:b + 1],
                                        ADD, SUB)
            if "l3f" not in abl:
                nc.sync.dma_start(
                    outp[:], obstore[:].rearrange("p b f -> p (b f)"))

    nc.compile()
    return nc


def _wrap16(a):
    """idx i -> partition i%16, col i//16; replicated to 128 partitions."""
    w = a.reshape(-1, 16).T
    return np.ascontiguousarray(np.tile(w, (8, 1)))


def _balanced_perm(deg):
    """Assign nodes to 80 blocks of 128 so block in-degree sums are even."""
    import heapq
    nblk = NPAD // P
    order = np.argsort(-deg, kind="stable")
    heap = [(0, 0, g) for g in range(nblk)]
    heapq.heapify(heap)
    newpos = np.empty(NPAD, np.int64)
    fill = np.zeros(nblk, np.int64)
    for n in order:
        s, _, g = heapq.heappop(heap)
        newpos[n] = g * P + fill[g]
        fill[g] += 1
        if fill[g] < P:
            heapq.heappush(heap, (s + int(deg[n]), int(fill[g]), g))
    return newpos


def _prep(x, edge_index):
    src = np.asarray(edge_index[0], dtype=np.int64)
    dst = np.asarray(edge_index[1], dtype=np.int64)
    deg = np.bincount(dst, minlength=NPAD).astype(np.float64)
    invdeg_n = (1.0 / np.maximum(deg, 1.0)).astype(np.float32)

    newpos = _balanced_perm(deg)
    oldnode = np.empty(NPAD, np.int64)
    oldnode[newpos] = np.arange(NPAD)
    psrc = newpos[src]
    pdst = newpos[dst]
    ivd_row = np.zeros(NPAD, np.float32)
    ivd_row[newpos] = invdeg_n

    # ---- L1 pull tables ----
    order = np.argsort(pdst, kind="stable")
    dsts = pdst[order]
    srcs = psrc[order]
    inv_e = ivd_row[dsts]
    starts = np.searchsorted(dsts, np.arange(0, NPAD + P, P))
    cnt = starts[1:] - starts[:-1]
    MC = max(1, int(np.ceil(cnt.max() / P)))
    per_core_l1 = []
    for r in range(NCORES):
        gparts, dparts, iparts = [], [], []
        for j in range(NB):
            g = r * NB + j
            lo, hi = starts[g], starts[g + 1]
            n = hi - lo
            o2 = lo + np.argsort(srcs[lo:hi], kind="stable")
            sg = np.zeros(MC * P, dtype=np.int16)
            dg = np.full(MC * P, -1.0, dtype=np.float32)
            ig = np.zeros(MC * P, dtype=np.float32)
            sg[:n] = srcs[o2].astype(np.int16)
            dg[:n] = (dsts[o2] - g * P).astype(np.float32)
            ig[:n] = inv_e[o2]
            gparts.append(_wrap16(sg))
            dparts.append(np.ascontiguousarray(dg.reshape(MC, P).T))
            iparts.append(np.ascontiguousarray(ig.reshape(MC, P).T))
        per_core_l1.append((
            np.concatenate(gparts, axis=1),
            np.concatenate(dparts, axis=1),
            np.concatenate(iparts, axis=1),
        ))

    # ---- push tables: per core, local out-edges chunked by dst block ----
    ecore = psrc // PER_CORE
    eloc = psrc % PER_CORE
    per_core_push = []
    blk_cnt = np.zeros((NCORES, NBLK), np.int64)
    for r in range(NCORES):
        m = ecore == r
        es, ed = eloc[m], pdst[m]
        o = np.argsort(ed, kind="stable")
        es, ed = es[o], ed[o]
        per_core_push.append((es, ed))
        blk_cnt[r] = np.bincount(ed // P, minlength=NBLK)
    CB = tuple(max(1, int(-(-blk_cnt[:, g].max() // P)))
               for g in range(NBLK))
    NCHP = sum(CB)
    offs = np.concatenate([[0], np.cumsum(CB)]).astype(int)

    push_tbl = []
    for r in range(NCORES):
        es, ed = per_core_push[r]
        bstart = np.searchsorted(ed // P, np.arange(NBLK + 1))
        sg = np.zeros(NCHP * P, np.int16)
        dgi = np.full(NCHP * P, -1.0, np.float32)
        ig = np.zeros(NCHP * P, np.float32)
        for g in range(NBLK):
            lo, hi = bstart[g], bstart[g + 1]
            n = hi - lo
            e0 = offs[g] * P
            sg[e0:e0 + n] = es[lo:hi].astype(np.int16)
            dgi[e0:e0 + n] = (ed[lo:hi] - g * P).astype(np.float32)
            ig[e0:e0 + n] = ivd_row[ed[lo:hi]]
        push_tbl.append(dict(
            g2t=_wrap16(sg),
            dstp=np.ascontiguousarray(dgi.reshape(NCHP, P).T),
            invp=np.ascontiguousarray(ig.reshape(NCHP, P).T)))

    xp = np.zeros((NPAD, D_IN), dtype=np.float32)
    xp[:N_NODES] = x
    xp = xp[oldnode]           # permuted node order

    return xp, per_core_l1, push_tbl, MC, CB, newpos


def _make_in_maps(x, edge_index, w1l, w1r, b1, w2l, w2r, b2, w3l, w3r, b3):
    x = np.ascontiguousarray(np.asarray(x, dtype=np.float32))
    xp, per_core_l1, push_tbl, MC, CB, newpos = _prep(
        x, np.asarray(edge_index))

    iota = np.tile(np.arange(P, dtype=np.float32), (P, 1))
    b1v = np.asarray(b1, np.float32).reshape(-1)
    b2v = np.asarray(b2, np.float32).reshape(-1)
    xbf = xp.astype(BF)
    common = {
        "xbf": xbf,
        "w1l": np.asarray(w1l, np.float32).astype(BF),
        "w1r": np.asarray(w1r, np.float32).astype(BF),
        "b1": b1v.reshape(1, D_H1).astype(BF),
        "b1t": np.ascontiguousarray(b1v.reshape(2, P).T),
        "w2l": np.asarray(w2l, np.float32).astype(BF),
        "w2r": np.asarray(w2r, np.float32).astype(BF),
        "b2t": np.ascontiguousarray(b2v.reshape(8, P).T),
        "w3lr": np.ascontiguousarray(np.concatenate(
            [np.asarray(w3l, np.float32), np.asarray(w3r, np.float32)],
            axis=1)).astype(BF),
        "b3pad": np.concatenate(
            [np.zeros(D_OUT, np.float32),
             np.asarray(b3, np.float32).reshape(-1)]).reshape(1, P).astype(BF),
        "iota_in": iota.astype(BF),
        "ident_in": np.eye(P, dtype=BF),
    }
    in_maps = []
    for r in range(NCORES):
        g, d, iv = per_core_l1[r]
        m = dict(common)
        m["xownT"] = np.ascontiguousarray(
            xbf[r * PER_CORE:(r + 1) * PER_CORE].T)
        m["gidx"] = g
        m["dstloc"] = d
        m["invdeg"] = iv
        m.update(push_tbl[r])
        in_maps.append(m)
    return in_maps, (MC, CB), newpos


def kernel(x, edge_index, w1l, w1r, b1, w2l, w2r, b2, w3l, w3r, b3):
    global LAST_RESULTS
    import os
    from concourse.bass_utils import run_bass_kernel_spmd

    if os.environ.get("BASS_TRACE"):
        try:
            import antenv.axon_hooks  # noqa: F401
        except ImportError:
            os.environ.pop("BASS_TRACE", None)  # no NTFF hook here

    in_maps, key, newpos = _make_in_maps(
        x, edge_index, w1l, w1r, b1, w2l, w2r, b2, w3l, w3r, b3)
    if key not in _CACHE:
        _CACHE[key] = _build(*key)
    nc = _CACHE[key]

    res = run_bass_kernel_spmd(nc, in_maps, core_ids=list(range(NCORES)))
    LAST_RESULTS = res
    out = np.concatenate(
        [res.results[r]["out"].reshape(P, NB, D_OUT).transpose(1, 0, 2)
         .reshape(PER_CORE, D_OUT) for r in range(NCORES)], axis=0)
    return np.ascontiguousarray(out[newpos[:N_NODES]])
